# revision 14
# baseline (speedup 1.0000x reference)
"""Trainium2 Bass kernel for nn_CheiralityLayer (cheirality loss) — v3.

Reference (per batch element b):
  gray = mean(img_pair[b, :3], axis=0)                       # [H, W]
  gx[h,w] = gray[h,w+1] - gray[h,w-1]   (zero padded)
  gy[h,w] = gray[h+1,w] - gray[h-1,w]   (zero padded)
  n = sqrt(gx^2 + gy^2 + 1e-8)
  P = gx*(x*V2 - V0) + gy*(y*V2 - V1)
  R = gx*(W0*x*y - W1*(x^2+1) + W2*y) + gy*(W0*(y^2+1) - W1*x*y - W2*x)
  rho = (P/n) * (nf0 + nf1 - R/n)
  out = mean(gelu(-rho))   (exact erf gelu)

v3 strategy (2 images/core, 4 row-bands of 120 rows each), engine-balanced
so every engine stays under the ~4.3us/tile DMA rate:
- gray_ext (122 rows: band + 1-row halo each side, halo rows in partitions
  120/121 so gx reads start at partition 0) via 6 f32r PE matmuls; single
  ACT drain to f16 graypad.
- gy via 2 f16 PE matmuls off graypad (cheaper than 6 f32r off raw img).
- gx (column stencil) + gx^2 on the otherwise idle Pool engine.
- gy^2 via ACT Square directly from PSUM (no gy drain); 1/n via ACT
  Abs_reciprocal_sqrt in ONE op (no DVE reciprocal); both live in the same
  act table as Copy/Square so the body needs a single table load.
- I1/tXn intermediate of v2 replaced by explicit x2gxn/xgyn products:
  drops the extra PSUM group (8 banks exactly) and a PSUM round-trip.
- v group absorbs nf0+nf1 via two identity-stationary f32r passes (PE has
  slack; saves a 1.4us Pool add).
- rho = Pdrain(ACT) * v_psum on DVE; per-core gelu+reduce tail as in v2.
- y-affine diag stationaries: image-0 set built on DVE during the initial
  DMA window; image-1 set prebuilt on host and DMA'd (overlapped).
"""

import numpy as np

B, C, H, W = 16, 6, 480, 640
NCORES = 8
BPC = B // NCORES          # images per core
NPOS = 4                   # row bands per image
NT = BPC * NPOS            # tiles per core
TH = 120                   # output rows per band
LR = 122                   # loaded img rows per band (gray rows -1..120)
RS = [0, 119, 239, 358]    # first loaded img row per band (clamped)
NSPLIT = [(0, 320), (320, 640)]
EPS = 9e-8                 # 9x reference eps (gray unscaled by 3)

NBLK = 23                  # const blocks per image: 4 bands x 5 diag + 3 sid
USE_ARS = True             # Abs_reciprocal_sqrt on ACT vs recip(DVE)+sqrt

_CACHE = {}


def _build_program(check_mode=False):
    """check_mode: skip the gelu+reduce tail (CoreSim lacks Gelu) and DMA
    per-pixel rho out as [TH, NT*W] f32 for numerical validation."""
    import concourse.bacc as bacc
    import concourse.tile as tile
    import concourse.mybir as mybir
    from contextlib import ExitStack

    f32 = mybir.dt.float32
    f32r = mybir.dt.float32r
    bf16 = mybir.dt.bfloat16
    f16 = mybir.dt.float16
    i16 = mybir.dt.int16
    AF = mybir.ActivationFunctionType
    OP = mybir.AluOpType

    nc = bacc.Bacc(
        "TRN2", target_bir_lowering=False, debug=False, enable_asserts=False
    )

    img_d = nc.dram_tensor("img3", [BPC, 3, H, W], f32r, kind="ExternalInput").ap()
    nf_d = nc.dram_tensor("nf", [BPC, 2, H, W], f32r, kind="ExternalInput").ap()
    cstb_d = nc.dram_tensor("cstb", [LR, NPOS * LR], f32r, kind="ExternalInput").ap()
    cstd_d = nc.dram_tensor("cstd", [LR, NPOS * TH], f16, kind="ExternalInput").ap()
    cstv_d = nc.dram_tensor("cstv", [128, 2 * NBLK], f32, kind="ExternalInput").ap()
    dgt1_d = nc.dram_tensor("dgt1", [TH, NBLK * TH], bf16, kind="ExternalInput").ap()
    if check_mode:
        rho_d = nc.dram_tensor(
            "rho_dbg", [TH, NT * W], f32, kind="ExternalOutput"
        ).ap()
    out_d = nc.dram_tensor("out", [1, 1], f32, kind="ExternalOutput").ap()

    def half(x):
        """[P, 640] AP -> [P, 2, 320] view matching psum halves."""
        return x.rearrange("p (b c) -> p b c", b=2)

    with tile.TileContext(nc) as tc, ExitStack() as ctx:
        consts = ctx.enter_context(tc.tile_pool(name="consts", bufs=1))
        imgp = ctx.enter_context(tc.tile_pool(name="imgp", bufs=5))
        nfp = ctx.enter_context(tc.tile_pool(name="nfp", bufs=6))
        work = ctx.enter_context(tc.tile_pool(name="work", bufs=3))
        psum = ctx.enter_context(tc.tile_pool(name="psum", bufs=1, space="PSUM"))

        # --- small constants first on the DMA queue ---
        cstb = consts.tile([LR, NPOS * LR], f32r)
        nc.sync.dma_start(cstb, cstb_d)
        cstd = consts.tile([LR, NPOS * TH], f16)
        nc.sync.dma_start(cstd, cstd_d)
        cstv = consts.tile([128, 2 * NBLK], f32)
        nc.sync.dma_start(cstv, cstv_d)

        def gmat(p):
            return cstb[0:LR, p * LR : (p + 1) * LR]

        def dmat(p):
            return cstd[0:LR, p * TH : (p + 1) * TH]

        # --- on-chip builds (overlap the initial DMA window) ---
        # x-coordinate row (0..639) on every partition, f16 (exact ints)
        xio = consts.tile([128, W], i16)
        nc.gpsimd.iota(xio, [[1, W]], base=0, channel_multiplier=0)
        X16 = consts.tile([128, W], f16)
        nc.vector.tensor_copy(X16, xio)
        X = X16[0:TH, :]

        # identity mask: iota(i - p) == 0
        io16 = consts.tile([TH, TH], i16)
        nc.gpsimd.iota(io16, [[1, TH]], base=0, channel_multiplier=-1)
        mask = consts.tile([TH, TH], bf16)
        nc.vector.tensor_scalar(mask, io16, 0, None, OP.is_equal)
        mask32 = consts.tile([TH, TH], f32r)
        nc.vector.tensor_copy(mask32, mask)

        # diag/sid stationaries: image 0 built on DVE, image 1 DMA'd below
        dgt = consts.tile([TH, 2 * NBLK * TH], bf16)
        for blk in range(NBLK):
            nc.vector.tensor_scalar_mul(
                dgt[:, blk * TH : (blk + 1) * TH], mask, cstv[0:TH, blk : blk + 1]
            )

        def dg(i, p, k):
            blk = i * NBLK + p * 5 + k
            return dgt[0:TH, blk * TH : (blk + 1) * TH]

        def sid(i, s):
            blk = i * NBLK + 20 + s
            return dgt[0:TH, blk * TH : (blk + 1) * TH]

        acc = consts.tile([128, 2], f32)
        nc.vector.memset(acc, 0.0)
        ones_t = consts.tile([128, 1], f32)
        nc.vector.memset(ones_t, 1.0)
        epsb = consts.tile([128, 1], f32)
        nc.vector.memset(epsb, EPS)

        rho_all = consts.tile([TH, NT * W], bf16)
        gelu_out = consts.tile([TH, NT * W], bf16)

        def front(t):
            i, p = divmod(t, NPOS)
            # img tile is column-padded: col 0 and col 641 stay zero so the
            # two overlapped gray halves cover gray cols -1..640 directly
            imgt = imgp.tile([LR, 3, W + 2], f32r, tag="imgt")
            if t < 5:  # imgp has 5 rotating buffers; pad cols stay zero
                nc.vector.memset(imgt[:, :, 0:1].bitcast(f32), 0.0)
                nc.vector.memset(imgt[:, :, W + 1 : W + 2].bitcast(f32), 0.0)
            nc.sync.dma_start(
                imgt[:, :, 1 : W + 1],
                img_d[i, :, RS[p] : RS[p] + LR, :].rearrange("c h w -> h c w"),
            )
            nft = nfp.tile([TH, 2, W], f32r, tag="nft")
            nc.sync.dma_start(
                nft,
                nf_d[i, :, TH * p : TH * (p + 1), :].rearrange("c h w -> h c w"),
            )
            return (imgt, nft)

        def grayblock(t, ft):
            imgt, nft = ft
            i, p = divmod(t, NPOS)
            # gray_ext, halo rows at partitions 120/121; halves overlap by 2
            # (fp32r moving size must be even): A = gray -1..320, B = 319..640
            gray_ps = psum.tile([LR, 2, 512], f32, tag="gray")
            for c3 in range(3):
                for b in range(2):
                    nc.tensor.matmul(
                        gray_ps[:, b, 0:322],
                        gmat(p),
                        imgt[:, c3, b * 320 : b * 320 + 322],
                        start=(c3 == 0),
                        stop=(c3 == 2),
                    )
            # two-part drain produces the zero-padded graypad [-1..640]
            graypad = work.tile([LR, W + 2], f16, tag="graypad")
            nc.scalar.copy(graypad[:, 0:322], gray_ps[:, 0, 0:322])
            nc.scalar.copy(graypad[:, 322 : W + 2], gray_ps[:, 1, 2:322])
            # gx: 16-bit column-shift subtract on DVE (rows 0..119 = band)
            gx = work.tile([TH, W], f16, tag="gx")
            nc.vector.tensor_sub(
                gx, graypad[0:TH, 2 : W + 2], graypad[0:TH, 0:W]
            )
            return (imgt, nft, graypad, gx)

        def gymm(t, gt):
            imgt, nft, graypad, gx = gt
            i, p = divmod(t, NPOS)
            # gy (row stencil) on PE off f16 graypad
            gy_ps = psum.tile([TH, 2, 512], f32, tag="gy")
            for b, (n0, n1) in enumerate(NSPLIT):
                nc.tensor.matmul(
                    gy_ps[:, b, 0:320],
                    dmat(p),
                    graypad[:, 1 + n0 : 1 + n1],
                    start=True,
                    stop=True,
                )
            return (t, i, p, nft, gx, gy_ps)

        def stencil(sg):
            t, i, p, nft, gx, gy_ps = sg
            # gy drain on ACT (frees the gy banks); squares on DVE/Pool
            gyb = work.tile([TH, W], f16, tag="gyb")
            nc.scalar.copy(half(gyb), gy_ps[:, :, 0:320])
            gy2 = work.tile([TH, W], bf16, tag="gy2")
            nc.vector.tensor_mul(gy2, gyb, gyb)
            gx2 = work.tile([TH, W], bf16, tag="gx2")
            nc.gpsimd.tensor_mul(gx2, gx, gx)
            return (t, i, p, nft, gx, gx2, gyb, gy2)

        def normblock(ns):
            t, i, p, nft, gx, gx2, gyb, gy2 = ns
            n2 = work.tile([TH, W], bf16, tag="n2")
            nc.vector.tensor_add(n2, gx2, gy2)
            rinv = work.tile([TH, W], bf16, tag="rinv")
            if USE_ARS:
                # eps folded into the activation bias: 1/sqrt(n2 + EPS)
                nc.scalar.activation(
                    rinv, n2, AF.Abs_reciprocal_sqrt, bias=epsb[0:TH, :]
                )
            else:
                n2f = work.tile([TH, W], f32, tag="n2f")
                nc.vector.tensor_scalar_add(n2f, n2, EPS)
                inv2 = work.tile([TH, W], f32, tag="inv2")
                nc.vector.reciprocal_approx_fast(out=inv2, in_=n2f)
                nc.scalar.sqrt(rinv, inv2)
            return (t, i, p, nft, gx, gyb, rinv)

        def normprods(ns):
            t, i, p, nft, gx, gyb, rinv = ns
            gxn = work.tile([TH, W], bf16, tag="gxn")
            nc.vector.tensor_mul(gxn, gx, rinv)
            gyn = work.tile([TH, W], bf16, tag="gyn")
            nc.vector.tensor_mul(gyn, gyb, rinv)
            xgxn = work.tile([TH, W], bf16, tag="xgxn")
            nc.gpsimd.tensor_mul(xgxn, gxn, X)
            xgyn = work.tile([TH, W], bf16, tag="xgyn")
            nc.vector.tensor_mul(xgyn, gyn, X)
            x2gxn = work.tile([TH, W], bf16, tag="x2gxn")
            nc.vector.tensor_mul(x2gxn, xgxn, X)
            return (t, i, p, nft, gxn, gyn, xgxn, xgyn, x2gxn)

        def backblock(st):
            t, i, p, nft, gxn, gyn, xgxn, xgyn, x2gxn = st
            # P/n = V2*xgxn - V0*gxn + (V2 y - V1) gyn
            P_ps = psum.tile([TH, 2, 512], f32, tag="P")
            psrc = [(sid(i, 0), xgxn), (sid(i, 1), gxn), (dg(i, p, 0), gyn)]
            for k, (m, src) in enumerate(psrc):
                for b, (n0, n1) in enumerate(NSPLIT):
                    nc.tensor.matmul(
                        P_ps[:, b, 0:320],
                        m,
                        src[:, n0:n1],
                        start=(k == 0),
                        stop=(k == len(psrc) - 1),
                    )
            # v = nf0 + nf1 - R/n
            #   = nf0 + nf1 + W1*x2gxn - W0y*xgxn + (W1y+W2)*xgyn
            #     + (W1 - W2y)*gxn - W0(y^2+1)*gyn
            v_ps = psum.tile([TH, 2, 512], f32, tag="V")
            vsrc = [
                (mask32, nft[:, 0, :]),
                (mask32, nft[:, 1, :]),
                (sid(i, 2), x2gxn),
                (dg(i, p, 1), xgxn),
                (dg(i, p, 2), xgyn),
                (dg(i, p, 3), gxn),
                (dg(i, p, 4), gyn),
            ]
            for k, (m, src) in enumerate(vsrc):
                for b, (n0, n1) in enumerate(NSPLIT):
                    nc.tensor.matmul(
                        v_ps[:, b, 0:320],
                        m,
                        src[:, n0:n1],
                        start=(k == 0),
                        stop=(k == len(vsrc) - 1),
                    )
            Pb = work.tile([TH, W], bf16, tag="pb")
            nc.scalar.copy(half(Pb), P_ps[:, :, 0:320])
            return (t, Pb, v_ps)

        def rhoblock(bk):
            t, Pb, v_ps = bk
            rho = rho_all[0:TH, t * W : (t + 1) * W]
            nc.vector.tensor_mul(half(rho), half(Pb), v_ps[:, :, 0:320])
            if check_mode:
                rho32 = work.tile([TH, W], f32, tag="rho32")
                nc.vector.tensor_copy(rho32, rho)
                nc.sync.dma_start(rho_d[:, t * W : (t + 1) * W], rho32)

        # --- software pipeline, 5-stage skew ---
        # iter k: rho(k) | P/v+Pb(k) | norms(k+1) | stencil(k+2) |
        #         gray(k+3) | dma(k+4); emission order makes each engine's
        #         in-order queue hit ops whose deps complete just-in-time.
        GELU_SPLIT = 6  # tiles 0..5 gelu'd mid-flight, 6..7 in the tail
        fts, gts, sgs, nss, sts, bks = {}, {}, {}, {}, {}, {}
        fts[0] = front(0)
        fts[1] = front(1)
        # image-1 stationaries arrive behind the first two tiles' data
        nc.sync.dma_start(dgt[:, NBLK * TH : 2 * NBLK * TH], dgt1_d)
        fts[2] = front(2)
        fts[3] = front(3)
        for k in range(-4, NT):
            if 0 <= k + 2 < NT and (k + 2) in gts:
                sgs[k + 2] = gymm(k + 2, gts.pop(k + 2))
            if 0 <= k + 1 < NT and (k + 1) in nss:
                nss[k + 1] = normblock(nss[k + 1])
            if 0 <= k < NT:
                bks[k] = backblock(sts.pop(k))
            if 0 <= k + 1 < NT and (k + 1) in nss:
                sts[k + 1] = normprods(nss.pop(k + 1))
            if 0 <= k < NT:
                rhoblock(bks.pop(k))
            if k == GELU_SPLIT - 1 and not check_mode:
                nc.scalar.activation(
                    gelu_out[0:TH, 0 : GELU_SPLIT * W],
                    rho_all[0:TH, 0 : GELU_SPLIT * W],
                    AF.Gelu,
                    scale=-1.0,
                    accum_out=acc[0:TH, 0:1],
                )
            if k + 4 < NT and (k + 4) not in fts:
                fts[k + 4] = front(k + 4)
            if 0 <= k + 2 < NT and (k + 2) in sgs:
                nss[k + 2] = stencil(sgs.pop(k + 2))
            if 0 <= k + 3 < NT:
                gts[k + 3] = grayblock(k + 3, fts.pop(k + 3))

        # --- tail: gelu (remaining tiles) + reduce ---
        if not check_mode:
            nc.scalar.activation(
                gelu_out[0:TH, GELU_SPLIT * W :],
                rho_all[0:TH, GELU_SPLIT * W :],
                AF.Gelu,
                scale=-1.0,
                accum_out=acc[0:TH, 1:2],
            )
        accs = consts.tile([128, 1], f32)
        nc.vector.reduce_sum(
            accs[0:TH, :], acc[0:TH, 0:2], axis=mybir.AxisListType.X
        )
        out_ps = psum.tile([1, 1], f32, tag="gray")
        nc.tensor.matmul(
            out_ps, accs[0:TH, 0:1], ones_t[0:TH, :], start=True, stop=True
        )
        res = consts.tile([1, 1], f32)
        nc.scalar.copy(res, out_ps)
        nc.sync.dma_start(out_d, res)

    nc.compile()
    return nc


def _host_constants(pose_np):
    """Host-built constants. cstb/cstd shared; cstv/dgt1 per core."""
    import ml_dtypes

    # gmat_ext: img row -> gray_ext partition (cols 0..119 = band rows,
    # col 120 = halo row band_start-1, col 121 = halo row band_start+120)
    gmat = np.zeros((LR, NPOS, LR), np.float32)
    # dmat: graypad partition -> gy row: gy[j] = gray[j+1] - gray[j-1]
    dmat = np.zeros((LR, NPOS, TH), np.float32)
    for p in range(NPOS):
        for j in range(LR):
            g = TH * p + j if j < TH else (TH * p - 1 if j == TH else TH * p + TH)
            if 0 <= g <= H - 1:
                gmat[g - RS[p], p, j] = 1.0
        for j in range(TH):
            q_plus = j + 1 if j < TH - 1 else LR - 1
            q_minus = j - 1 if j >= 1 else TH
            dmat[q_plus, p, j] += 1.0
            dmat[q_minus, p, j] -= 1.0
    cstb = gmat.reshape(LR, -1)
    cstd = dmat.reshape(LR, -1).astype(np.float16)

    cstv_list = []
    dgt1_list = []
    jj = np.arange(TH, dtype=np.float32)
    for core in range(NCORES):
        vals = np.zeros((128, 2 * NBLK), np.float32)
        for i in range(BPC):
            b = core * BPC + i
            V0, V1, V2, W0, W1, W2 = [float(x) for x in pose_np[b]]
            base = i * NBLK
            for p in range(NPOS):
                yv = TH * p + jj
                o = base + p * 5
                vals[0:TH, o + 0] = V2 * yv - V1
                vals[0:TH, o + 1] = -W0 * yv
                vals[0:TH, o + 2] = W1 * yv + W2
                vals[0:TH, o + 3] = W1 - W2 * yv
                vals[0:TH, o + 4] = -W0 * (yv * yv + 1.0)
            vals[0:TH, base + 20] = V2
            vals[0:TH, base + 21] = -V0
            vals[0:TH, base + 22] = W1
        cstv_list.append(vals)
        dgt1 = np.zeros((TH, NBLK * TH), np.float32)
        for blk in range(NBLK):
            np.fill_diagonal(
                dgt1[:, blk * TH : (blk + 1) * TH], vals[0:TH, NBLK + blk]
            )
        dgt1_list.append(dgt1.astype(ml_dtypes.bfloat16))
    return cstb, cstd, cstv_list, dgt1_list


def kernel(img_pair, pose, normal_flow):
    from concourse.bass_utils import run_bass_kernel_spmd

    img_pair = np.asarray(img_pair, dtype=np.float32)
    pose = np.asarray(pose, dtype=np.float32)
    normal_flow = np.asarray(normal_flow, dtype=np.float32)

    if "nc" not in _CACHE:
        _CACHE["nc"] = _build_program()
    nc = _CACHE["nc"]

    cstb, cstd, cstv_list, dgt1_list = _host_constants(pose)
    in_maps = []
    for core in range(NCORES):
        b0 = core * BPC
        in_maps.append(
            {
                "img3": np.ascontiguousarray(img_pair[b0 : b0 + BPC, :3]),
                "nf": np.ascontiguousarray(normal_flow[b0 : b0 + BPC]),
                "cstb": cstb,
                "cstd": cstd,
                "cstv": cstv_list[core],
                "dgt1": dgt1_list[core],
            }
        )

    _CACHE["in_maps"] = in_maps
    res = run_bass_kernel_spmd(nc, in_maps, core_ids=list(range(NCORES)))
    total = np.float64(0.0)
    for r in res.results:
        total += np.float64(r["out"][0, 0])
    out = np.float32(total / (B * H * W))
    return np.asarray(out, dtype=np.float32)


# revision 16
# speedup vs baseline: 1.0788x; 1.0788x over previous
"""Trainium2 Bass kernel for nn_CheiralityLayer (cheirality loss) — v3.

Reference (per batch element b):
  gray = mean(img_pair[b, :3], axis=0)                       # [H, W]
  gx[h,w] = gray[h,w+1] - gray[h,w-1]   (zero padded)
  gy[h,w] = gray[h+1,w] - gray[h-1,w]   (zero padded)
  n = sqrt(gx^2 + gy^2 + 1e-8)
  P = gx*(x*V2 - V0) + gy*(y*V2 - V1)
  R = gx*(W0*x*y - W1*(x^2+1) + W2*y) + gy*(W0*(y^2+1) - W1*x*y - W2*x)
  rho = (P/n) * (nf0 + nf1 - R/n)
  out = mean(gelu(-rho))   (exact erf gelu)

v3 strategy (2 images/core, 4 row-bands of 120 rows each), engine-balanced
so every engine stays under the ~4.3us/tile DMA rate:
- gray_ext (122 rows: band + 1-row halo each side, halo rows in partitions
  120/121 so gx reads start at partition 0) via 6 f32r PE matmuls; single
  ACT drain to f16 graypad.
- gy via 2 f16 PE matmuls off graypad (cheaper than 6 f32r off raw img).
- gx (column stencil) + gx^2 on the otherwise idle Pool engine.
- gy^2 via ACT Square directly from PSUM (no gy drain); 1/n via ACT
  Abs_reciprocal_sqrt in ONE op (no DVE reciprocal); both live in the same
  act table as Copy/Square so the body needs a single table load.
- I1/tXn intermediate of v2 replaced by explicit x2gxn/xgyn products:
  drops the extra PSUM group (8 banks exactly) and a PSUM round-trip.
- v group absorbs nf0+nf1 via two identity-stationary f32r passes (PE has
  slack; saves a 1.4us Pool add).
- rho = Pdrain(ACT) * v_psum on DVE; per-core gelu+reduce tail as in v2.
- y-affine diag stationaries: image-0 set built on DVE during the initial
  DMA window; image-1 set prebuilt on host and DMA'd (overlapped).
"""

import numpy as np

B, C, H, W = 16, 6, 480, 640
NCORES = 8
BPC = B // NCORES          # images per core
NPOS = 4                   # row bands per image
NT = BPC * NPOS            # tiles per core
TH = 120                   # output rows per band
LR = 122                   # loaded img rows per band (gray rows -1..120)
RS = [0, 119, 239, 358]    # first loaded img row per band (clamped)
NSPLIT = [(0, 320), (320, 640)]
EPS = 9e-8                 # 9x reference eps (gray unscaled by 3)

NBLK = 23                  # const blocks per image: 4 bands x 5 diag + 3 sid
USE_ARS = True             # Abs_reciprocal_sqrt on ACT vs recip(DVE)+sqrt

_CACHE = {}


def _build_program(check_mode=False):
    """check_mode: skip the gelu+reduce tail (CoreSim lacks Gelu) and DMA
    per-pixel rho out as [TH, NT*W] f32 for numerical validation."""
    import concourse.bacc as bacc
    import concourse.tile as tile
    import concourse.mybir as mybir
    from contextlib import ExitStack

    f32 = mybir.dt.float32
    f32r = mybir.dt.float32r
    bf16 = mybir.dt.bfloat16
    f16 = mybir.dt.float16
    i16 = mybir.dt.int16
    AF = mybir.ActivationFunctionType
    OP = mybir.AluOpType

    nc = bacc.Bacc(
        "TRN2", target_bir_lowering=False, debug=False, enable_asserts=False
    )

    img_d = nc.dram_tensor("img3", [BPC, 3, H, W], f32r, kind="ExternalInput").ap()
    nf_d = nc.dram_tensor("nf", [BPC, 2, H, W], f32r, kind="ExternalInput").ap()
    cstb_d = nc.dram_tensor("cstb", [LR, NPOS * LR], f32r, kind="ExternalInput").ap()
    cstd_d = nc.dram_tensor("cstd", [LR, NPOS * TH], f16, kind="ExternalInput").ap()
    cstv_d = nc.dram_tensor("cstv", [128, 2 * NBLK], f32, kind="ExternalInput").ap()
    dgt1_d = nc.dram_tensor("dgt1", [TH, NBLK * TH], bf16, kind="ExternalInput").ap()
    if check_mode:
        rho_d = nc.dram_tensor(
            "rho_dbg", [TH, NT * W], f32, kind="ExternalOutput"
        ).ap()
    out_d = nc.dram_tensor("out", [1, 1], f32, kind="ExternalOutput").ap()

    def half(x):
        """[P, 640] AP -> [P, 2, 320] view matching psum halves."""
        return x.rearrange("p (b c) -> p b c", b=2)

    with tile.TileContext(nc) as tc, ExitStack() as ctx:
        consts = ctx.enter_context(tc.tile_pool(name="consts", bufs=1))
        imgp = ctx.enter_context(tc.tile_pool(name="imgp", bufs=5))
        nfp = ctx.enter_context(tc.tile_pool(name="nfp", bufs=6))
        work = ctx.enter_context(tc.tile_pool(name="work", bufs=3))
        psum = ctx.enter_context(tc.tile_pool(name="psum", bufs=1, space="PSUM"))

        # --- small constants first on the DMA queue ---
        cstb = consts.tile([LR, NPOS * LR], f32r)
        nc.sync.dma_start(cstb, cstb_d)
        cstd = consts.tile([LR, NPOS * TH], f16)
        nc.sync.dma_start(cstd, cstd_d)
        cstv = consts.tile([128, 2 * NBLK], f32)
        nc.sync.dma_start(cstv, cstv_d)

        def gmat(p):
            return cstb[0:LR, p * LR : (p + 1) * LR]

        def dmat(p):
            return cstd[0:LR, p * TH : (p + 1) * TH]

        # --- on-chip builds (overlap the initial DMA window) ---
        # x-coordinate row (0..639) on every partition, f16 (exact ints)
        xio = consts.tile([128, W], i16)
        nc.gpsimd.iota(xio, [[1, W]], base=0, channel_multiplier=0)
        X16 = consts.tile([128, W], f16)
        nc.vector.tensor_copy(X16, xio)
        X = X16[0:TH, :]

        # identity mask: iota(i - p) == 0
        io16 = consts.tile([TH, TH], i16)
        nc.gpsimd.iota(io16, [[1, TH]], base=0, channel_multiplier=-1)
        mask = consts.tile([TH, TH], bf16)
        nc.vector.tensor_scalar(mask, io16, 0, None, OP.is_equal)
        mask32 = consts.tile([TH, TH], f32r)
        nc.vector.tensor_copy(mask32, mask)

        # diag/sid stationaries: image 0 built on DVE, image 1 DMA'd below
        dgt = consts.tile([TH, 2 * NBLK * TH], bf16)
        for blk in range(NBLK):
            nc.vector.tensor_scalar_mul(
                dgt[:, blk * TH : (blk + 1) * TH], mask, cstv[0:TH, blk : blk + 1]
            )

        def dg(i, p, k):
            blk = i * NBLK + p * 5 + k
            return dgt[0:TH, blk * TH : (blk + 1) * TH]

        def sid(i, s):
            blk = i * NBLK + 20 + s
            return dgt[0:TH, blk * TH : (blk + 1) * TH]

        acc = consts.tile([128, 2], f32)
        nc.vector.memset(acc, 0.0)
        ones_t = consts.tile([128, 1], f32)
        nc.vector.memset(ones_t, 1.0)
        epsb = consts.tile([128, 1], f32)
        nc.vector.memset(epsb, EPS)

        rho_all = consts.tile([TH, NT * W], bf16)
        gelu_out = consts.tile([TH, NT * W], bf16)

        def front(t):
            i, p = divmod(t, NPOS)
            # img tile is column-padded: col 0 and col 641 stay zero so the
            # two overlapped gray halves cover gray cols -1..640 directly
            imgt = imgp.tile([LR, 3, W + 2], f32r, tag="imgt")
            if t < 5:  # imgp has 5 rotating buffers; pad cols stay zero
                nc.vector.memset(imgt[:, :, 0:1].bitcast(f32), 0.0)
                nc.vector.memset(imgt[:, :, W + 1 : W + 2].bitcast(f32), 0.0)
            nc.sync.dma_start(
                imgt[:, :, 1 : W + 1],
                img_d[i, :, RS[p] : RS[p] + LR, :].rearrange("c h w -> h c w"),
            )
            nft = nfp.tile([TH, 2, W], f32r, tag="nft")
            nc.sync.dma_start(
                nft,
                nf_d[i, :, TH * p : TH * (p + 1), :].rearrange("c h w -> h c w"),
            )
            return (imgt, nft)

        def grayblock(t, ft):
            imgt, nft = ft
            i, p = divmod(t, NPOS)
            # gray_ext, halo rows at partitions 120/121; halves overlap by 2
            # (fp32r moving size must be even): A = gray -1..320, B = 319..640
            gray_ps = psum.tile([LR, 2, 512], f32, tag="gray")
            for c3 in range(3):
                for b in range(2):
                    nc.tensor.matmul(
                        gray_ps[:, b, 0:322],
                        gmat(p),
                        imgt[:, c3, b * 320 : b * 320 + 322],
                        start=(c3 == 0),
                        stop=(c3 == 2),
                    )
            # two-part drain produces the zero-padded graypad [-1..640]
            graypad = work.tile([LR, W + 2], f16, tag="graypad")
            nc.scalar.copy(graypad[:, 0:322], gray_ps[:, 0, 0:322])
            nc.scalar.copy(graypad[:, 322 : W + 2], gray_ps[:, 1, 2:322])
            # gx: 16-bit column-shift subtract on DVE (rows 0..119 = band)
            gx = work.tile([TH, W], f16, tag="gx")
            nc.vector.tensor_sub(
                gx, graypad[0:TH, 2 : W + 2], graypad[0:TH, 0:W]
            )
            return (imgt, nft, graypad, gx)

        def gymm(t, gt):
            imgt, nft, graypad, gx = gt
            i, p = divmod(t, NPOS)
            # gy (row stencil) on PE off f16 graypad
            gy_ps = psum.tile([TH, 2, 512], f32, tag="gy")
            for b, (n0, n1) in enumerate(NSPLIT):
                nc.tensor.matmul(
                    gy_ps[:, b, 0:320],
                    dmat(p),
                    graypad[:, 1 + n0 : 1 + n1],
                    start=True,
                    stop=True,
                )
            return (t, i, p, nft, gx, gy_ps)

        def stencil(sg):
            t, i, p, nft, gx, gy_ps = sg
            # gy drain on ACT (frees the gy banks); squares on DVE/Pool
            gyb = work.tile([TH, W], f16, tag="gyb")
            nc.scalar.copy(half(gyb), gy_ps[:, :, 0:320])
            gy2 = work.tile([TH, W], bf16, tag="gy2")
            nc.vector.tensor_mul(gy2, gyb, gyb)
            gx2 = work.tile([TH, W], bf16, tag="gx2")
            nc.gpsimd.tensor_mul(gx2, gx, gx)
            return (t, i, p, nft, gx, gx2, gyb, gy2)

        def normblock(ns):
            t, i, p, nft, gx, gx2, gyb, gy2 = ns
            n2 = work.tile([TH, W], bf16, tag="n2")
            nc.vector.tensor_add(n2, gx2, gy2)
            rinv = work.tile([TH, W], bf16, tag="rinv")
            if USE_ARS:
                # eps folded into the activation bias: 1/sqrt(n2 + EPS)
                nc.scalar.activation(
                    rinv, n2, AF.Abs_reciprocal_sqrt, bias=epsb[0:TH, :]
                )
            else:
                n2f = work.tile([TH, W], f32, tag="n2f")
                nc.vector.tensor_scalar_add(n2f, n2, EPS)
                inv2 = work.tile([TH, W], f32, tag="inv2")
                nc.vector.reciprocal_approx_fast(out=inv2, in_=n2f)
                nc.scalar.sqrt(rinv, inv2)
            return (t, i, p, nft, gx, gyb, rinv)

        def normprods(ns):
            t, i, p, nft, gx, gyb, rinv = ns
            gxn = work.tile([TH, W], bf16, tag="gxn")
            nc.vector.tensor_mul(gxn, gx, rinv)
            gyn = work.tile([TH, W], bf16, tag="gyn")
            nc.vector.tensor_mul(gyn, gyb, rinv)
            xgxn = work.tile([TH, W], bf16, tag="xgxn")
            nc.vector.tensor_mul(xgxn, gxn, X)
            xgyn = work.tile([TH, W], bf16, tag="xgyn")
            nc.vector.tensor_mul(xgyn, gyn, X)
            x2gxn = work.tile([TH, W], bf16, tag="x2gxn")
            nc.vector.tensor_mul(x2gxn, xgxn, X)
            return (t, i, p, nft, gxn, gyn, xgxn, xgyn, x2gxn)

        def backblock(st):
            t, i, p, nft, gxn, gyn, xgxn, xgyn, x2gxn = st
            # P/n = V2*xgxn - V0*gxn + (V2 y - V1) gyn
            P_ps = psum.tile([TH, 2, 512], f32, tag="P")
            psrc = [(sid(i, 0), xgxn), (sid(i, 1), gxn), (dg(i, p, 0), gyn)]
            for k, (m, src) in enumerate(psrc):
                for b, (n0, n1) in enumerate(NSPLIT):
                    nc.tensor.matmul(
                        P_ps[:, b, 0:320],
                        m,
                        src[:, n0:n1],
                        start=(k == 0),
                        stop=(k == len(psrc) - 1),
                    )
            # v = nf0 + nf1 - R/n
            #   = nf0 + nf1 + W1*x2gxn - W0y*xgxn + (W1y+W2)*xgyn
            #     + (W1 - W2y)*gxn - W0(y^2+1)*gyn
            v_ps = psum.tile([TH, 2, 512], f32, tag="V")
            vsrc = [
                (mask32, nft[:, 0, :]),
                (mask32, nft[:, 1, :]),
                (sid(i, 2), x2gxn),
                (dg(i, p, 1), xgxn),
                (dg(i, p, 2), xgyn),
                (dg(i, p, 3), gxn),
                (dg(i, p, 4), gyn),
            ]
            for k, (m, src) in enumerate(vsrc):
                for b, (n0, n1) in enumerate(NSPLIT):
                    nc.tensor.matmul(
                        v_ps[:, b, 0:320],
                        m,
                        src[:, n0:n1],
                        start=(k == 0),
                        stop=(k == len(vsrc) - 1),
                    )
            Pb = work.tile([TH, W], bf16, tag="pb")
            nc.scalar.copy(half(Pb), P_ps[:, :, 0:320])
            return (t, Pb, v_ps)

        def rhoblock(bk):
            t, Pb, v_ps = bk
            rho = rho_all[0:TH, t * W : (t + 1) * W]
            nc.vector.tensor_mul(half(rho), half(Pb), v_ps[:, :, 0:320])
            if check_mode:
                rho32 = work.tile([TH, W], f32, tag="rho32")
                nc.vector.tensor_copy(rho32, rho)
                nc.sync.dma_start(rho_d[:, t * W : (t + 1) * W], rho32)

        # --- software pipeline, 5-stage skew ---
        # iter k: rho(k) | P/v+Pb(k) | norms(k+1) | stencil(k+2) |
        #         gray(k+3) | dma(k+4); emission order makes each engine's
        #         in-order queue hit ops whose deps complete just-in-time.
        GELU_SPLIT = 6  # tiles 0..5 gelu'd mid-flight, 6..7 in the tail
        fts, gts, sgs, nss, sts, bks = {}, {}, {}, {}, {}, {}
        fts[0] = front(0)
        fts[1] = front(1)
        # image-1 stationaries arrive behind the first two tiles' data
        nc.sync.dma_start(dgt[:, NBLK * TH : 2 * NBLK * TH], dgt1_d)
        fts[2] = front(2)
        fts[3] = front(3)
        for k in range(-4, NT):
            if 0 <= k + 2 < NT and (k + 2) in gts:
                sgs[k + 2] = gymm(k + 2, gts.pop(k + 2))
            if 0 <= k + 1 < NT and (k + 1) in nss:
                nss[k + 1] = normblock(nss[k + 1])
            if 0 <= k < NT:
                bks[k] = backblock(sts.pop(k))
            if 0 <= k + 1 < NT and (k + 1) in nss:
                sts[k + 1] = normprods(nss.pop(k + 1))
            if 0 <= k < NT:
                rhoblock(bks.pop(k))
            if k == NT - 2 and not check_mode:
                # all ars-table uses (rinv of every tile) are now emitted;
                # switching to the gelu table here costs no extra reload
                nc.scalar.activation(
                    gelu_out[0:TH, 0 : GELU_SPLIT * W],
                    rho_all[0:TH, 0 : GELU_SPLIT * W],
                    AF.Gelu,
                    scale=-1.0,
                    accum_out=acc[0:TH, 0:1],
                )
            if k + 4 < NT and (k + 4) not in fts:
                fts[k + 4] = front(k + 4)
            if 0 <= k + 2 < NT and (k + 2) in sgs:
                nss[k + 2] = stencil(sgs.pop(k + 2))
            if 0 <= k + 3 < NT:
                gts[k + 3] = grayblock(k + 3, fts.pop(k + 3))

        # --- tail: gelu (remaining tiles) + reduce ---
        if not check_mode:
            nc.scalar.activation(
                gelu_out[0:TH, GELU_SPLIT * W :],
                rho_all[0:TH, GELU_SPLIT * W :],
                AF.Gelu,
                scale=-1.0,
                accum_out=acc[0:TH, 1:2],
            )
        accs = consts.tile([128, 1], f32)
        nc.vector.reduce_sum(
            accs[0:TH, :], acc[0:TH, 0:2], axis=mybir.AxisListType.X
        )
        out_ps = psum.tile([1, 1], f32, tag="gray")
        nc.tensor.matmul(
            out_ps, accs[0:TH, 0:1], ones_t[0:TH, :], start=True, stop=True
        )
        res = consts.tile([1, 1], f32)
        nc.scalar.copy(res, out_ps)
        nc.sync.dma_start(out_d, res)

    nc.compile()
    return nc


def _host_constants(pose_np):
    """Host-built constants. cstb/cstd shared; cstv/dgt1 per core."""
    import ml_dtypes

    # gmat_ext: img row -> gray_ext partition (cols 0..119 = band rows,
    # col 120 = halo row band_start-1, col 121 = halo row band_start+120)
    gmat = np.zeros((LR, NPOS, LR), np.float32)
    # dmat: graypad partition -> gy row: gy[j] = gray[j+1] - gray[j-1]
    dmat = np.zeros((LR, NPOS, TH), np.float32)
    for p in range(NPOS):
        for j in range(LR):
            g = TH * p + j if j < TH else (TH * p - 1 if j == TH else TH * p + TH)
            if 0 <= g <= H - 1:
                gmat[g - RS[p], p, j] = 1.0
        for j in range(TH):
            q_plus = j + 1 if j < TH - 1 else LR - 1
            q_minus = j - 1 if j >= 1 else TH
            dmat[q_plus, p, j] += 1.0
            dmat[q_minus, p, j] -= 1.0
    cstb = gmat.reshape(LR, -1)
    cstd = dmat.reshape(LR, -1).astype(np.float16)

    cstv_list = []
    dgt1_list = []
    jj = np.arange(TH, dtype=np.float32)
    for core in range(NCORES):
        vals = np.zeros((128, 2 * NBLK), np.float32)
        for i in range(BPC):
            b = core * BPC + i
            V0, V1, V2, W0, W1, W2 = [float(x) for x in pose_np[b]]
            base = i * NBLK
            for p in range(NPOS):
                yv = TH * p + jj
                o = base + p * 5
                vals[0:TH, o + 0] = V2 * yv - V1
                vals[0:TH, o + 1] = -W0 * yv
                vals[0:TH, o + 2] = W1 * yv + W2
                vals[0:TH, o + 3] = W1 - W2 * yv
                vals[0:TH, o + 4] = -W0 * (yv * yv + 1.0)
            vals[0:TH, base + 20] = V2
            vals[0:TH, base + 21] = -V0
            vals[0:TH, base + 22] = W1
        cstv_list.append(vals)
        dgt1 = np.zeros((TH, NBLK * TH), np.float32)
        for blk in range(NBLK):
            np.fill_diagonal(
                dgt1[:, blk * TH : (blk + 1) * TH], vals[0:TH, NBLK + blk]
            )
        dgt1_list.append(dgt1.astype(ml_dtypes.bfloat16))
    return cstb, cstd, cstv_list, dgt1_list


def kernel(img_pair, pose, normal_flow):
    from concourse.bass_utils import run_bass_kernel_spmd

    img_pair = np.asarray(img_pair, dtype=np.float32)
    pose = np.asarray(pose, dtype=np.float32)
    normal_flow = np.asarray(normal_flow, dtype=np.float32)

    if "nc" not in _CACHE:
        _CACHE["nc"] = _build_program()
    nc = _CACHE["nc"]

    cstb, cstd, cstv_list, dgt1_list = _host_constants(pose)
    in_maps = []
    for core in range(NCORES):
        b0 = core * BPC
        in_maps.append(
            {
                "img3": np.ascontiguousarray(img_pair[b0 : b0 + BPC, :3]),
                "nf": np.ascontiguousarray(normal_flow[b0 : b0 + BPC]),
                "cstb": cstb,
                "cstd": cstd,
                "cstv": cstv_list[core],
                "dgt1": dgt1_list[core],
            }
        )

    _CACHE["in_maps"] = in_maps
    res = run_bass_kernel_spmd(nc, in_maps, core_ids=list(range(NCORES)))
    total = np.float64(0.0)
    for r in res.results:
        total += np.float64(r["out"][0, 0])
    out = np.float32(total / (B * H * W))
    return np.asarray(out, dtype=np.float32)


# revision 18
# speedup vs baseline: 1.1564x; 1.0719x over previous
"""Trainium2 Bass kernel for nn_CheiralityLayer (cheirality loss) — v3.

Reference (per batch element b):
  gray = mean(img_pair[b, :3], axis=0)                       # [H, W]
  gx[h,w] = gray[h,w+1] - gray[h,w-1]   (zero padded)
  gy[h,w] = gray[h+1,w] - gray[h-1,w]   (zero padded)
  n = sqrt(gx^2 + gy^2 + 1e-8)
  P = gx*(x*V2 - V0) + gy*(y*V2 - V1)
  R = gx*(W0*x*y - W1*(x^2+1) + W2*y) + gy*(W0*(y^2+1) - W1*x*y - W2*x)
  rho = (P/n) * (nf0 + nf1 - R/n)
  out = mean(gelu(-rho))   (exact erf gelu)

v3 strategy (2 images/core, 4 row-bands of 120 rows each), engine-balanced
so every engine stays under the ~4.3us/tile DMA rate:
- gray_ext (122 rows: band + 1-row halo each side, halo rows in partitions
  120/121 so gx reads start at partition 0) via 6 f32r PE matmuls; single
  ACT drain to f16 graypad.
- gy via 2 f16 PE matmuls off graypad (cheaper than 6 f32r off raw img).
- gx (column stencil) + gx^2 on the otherwise idle Pool engine.
- gy^2 via ACT Square directly from PSUM (no gy drain); 1/n via ACT
  Abs_reciprocal_sqrt in ONE op (no DVE reciprocal); both live in the same
  act table as Copy/Square so the body needs a single table load.
- I1/tXn intermediate of v2 replaced by explicit x2gxn/xgyn products:
  drops the extra PSUM group (8 banks exactly) and a PSUM round-trip.
- v group absorbs nf0+nf1 via two identity-stationary f32r passes (PE has
  slack; saves a 1.4us Pool add).
- rho = Pdrain(ACT) * v_psum on DVE; per-core gelu+reduce tail as in v2.
- y-affine diag stationaries: image-0 set built on DVE during the initial
  DMA window; image-1 set prebuilt on host and DMA'd (overlapped).
"""

import numpy as np

B, C, H, W = 16, 6, 480, 640
NCORES = 8
BPC = B // NCORES          # images per core
NPOS = 4                   # row bands per image
NT = BPC * NPOS            # tiles per core
TH = 120                   # output rows per band
LR = 122                   # loaded img rows per band (gray rows -1..120)
RS = [0, 119, 239, 358]    # first loaded img row per band (clamped)
NSPLIT = [(0, 320), (320, 640)]
EPS = 9e-8                 # 9x reference eps (gray unscaled by 3)

NBLK = 23                  # const blocks per image: 4 bands x 5 diag + 3 sid
USE_ARS = True             # Abs_reciprocal_sqrt on ACT vs recip(DVE)+sqrt

_CACHE = {}


def _build_program(check_mode=False):
    """check_mode: skip the gelu+reduce tail (CoreSim lacks Gelu) and DMA
    per-pixel rho out as [TH, NT*W] f32 for numerical validation."""
    import concourse.bacc as bacc
    import concourse.tile as tile
    import concourse.mybir as mybir
    from contextlib import ExitStack

    f32 = mybir.dt.float32
    f32r = mybir.dt.float32r
    bf16 = mybir.dt.bfloat16
    f16 = mybir.dt.float16
    i16 = mybir.dt.int16
    AF = mybir.ActivationFunctionType
    OP = mybir.AluOpType

    nc = bacc.Bacc(
        "TRN2", target_bir_lowering=False, debug=False, enable_asserts=False
    )

    img_d = nc.dram_tensor("img3", [BPC, 3, H, W], f32r, kind="ExternalInput").ap()
    nf_d = nc.dram_tensor("nf", [BPC, 2, H, W], f32r, kind="ExternalInput").ap()
    cstb_d = nc.dram_tensor("cstb", [LR, NPOS * LR], f32r, kind="ExternalInput").ap()
    cstd_d = nc.dram_tensor("cstd", [LR, NPOS * TH], f16, kind="ExternalInput").ap()
    cstv_d = nc.dram_tensor("cstv", [128, 2 * NBLK], f32, kind="ExternalInput").ap()
    dgt1_d = nc.dram_tensor("dgt1", [TH, NBLK * TH], bf16, kind="ExternalInput").ap()
    if check_mode:
        rho_d = nc.dram_tensor(
            "rho_dbg", [TH, NT * W], f32, kind="ExternalOutput"
        ).ap()
    out_d = nc.dram_tensor("out", [1, 1], f32, kind="ExternalOutput").ap()

    def half(x):
        """[P, 640] AP -> [P, 2, 320] view matching psum halves."""
        return x.rearrange("p (b c) -> p b c", b=2)

    with tile.TileContext(nc) as tc, ExitStack() as ctx:
        consts = ctx.enter_context(tc.tile_pool(name="consts", bufs=1))
        imgp = ctx.enter_context(tc.tile_pool(name="imgp", bufs=5))
        nfp = ctx.enter_context(tc.tile_pool(name="nfp", bufs=6))
        work = ctx.enter_context(tc.tile_pool(name="work", bufs=3))
        psum = ctx.enter_context(tc.tile_pool(name="psum", bufs=1, space="PSUM"))

        # --- small constants first on the DMA queue ---
        cstb = consts.tile([LR, NPOS * LR], f32r)
        nc.sync.dma_start(cstb, cstb_d)
        cstd = consts.tile([LR, NPOS * TH], f16)
        nc.sync.dma_start(cstd, cstd_d)
        cstv = consts.tile([128, 2 * NBLK], f32)
        nc.sync.dma_start(cstv, cstv_d)

        def gmat(p):
            return cstb[0:LR, p * LR : (p + 1) * LR]

        def dmat(p):
            return cstd[0:LR, p * TH : (p + 1) * TH]

        # --- on-chip builds (overlap the initial DMA window) ---
        # x-coordinate row (0..639) on every partition, f16 (exact ints)
        xio = consts.tile([128, W], i16)
        nc.gpsimd.iota(xio, [[1, W]], base=0, channel_multiplier=0)
        X16 = consts.tile([128, W], f16)
        nc.vector.tensor_copy(X16, xio)
        X = X16[0:TH, :]

        # identity mask: iota(i - p) == 0
        io16 = consts.tile([TH, TH], i16)
        nc.gpsimd.iota(io16, [[1, TH]], base=0, channel_multiplier=-1)
        mask = consts.tile([TH, TH], bf16)
        nc.vector.tensor_scalar(mask, io16, 0, None, OP.is_equal)
        mask32 = consts.tile([TH, TH], f32r)
        nc.vector.tensor_copy(mask32, mask)

        # diag/sid stationaries: image 0 built on DVE, image 1 DMA'd below
        dgt = consts.tile([TH, 2 * NBLK * TH], bf16)
        for blk in range(NBLK):
            nc.vector.tensor_scalar_mul(
                dgt[:, blk * TH : (blk + 1) * TH], mask, cstv[0:TH, blk : blk + 1]
            )

        def dg(i, p, k):
            blk = i * NBLK + p * 5 + k
            return dgt[0:TH, blk * TH : (blk + 1) * TH]

        def sid(i, s):
            blk = i * NBLK + 20 + s
            return dgt[0:TH, blk * TH : (blk + 1) * TH]

        acc = consts.tile([128, 2], f32)
        nc.vector.memset(acc, 0.0)
        ones_t = consts.tile([128, 1], f32)
        nc.vector.memset(ones_t, 1.0)
        epsb = consts.tile([128, 1], f32)
        nc.vector.memset(epsb, EPS)

        rho_all = consts.tile([TH, NT * W], bf16)
        gelu_out = consts.tile([TH, NT * W], bf16)

        def front(t):
            i, p = divmod(t, NPOS)
            # img tile is column-padded: col 0 and col 641 stay zero so the
            # two overlapped gray halves cover gray cols -1..640 directly
            imgt = imgp.tile([LR, 3, W + 2], f32r, tag="imgt")
            if t < 5:  # imgp has 5 rotating buffers; pad cols stay zero
                nc.vector.memset(imgt[:, :, 0:1].bitcast(f32), 0.0)
                nc.vector.memset(imgt[:, :, W + 1 : W + 2].bitcast(f32), 0.0)
            nc.sync.dma_start(
                imgt[:, :, 1 : W + 1],
                img_d[i, :, RS[p] : RS[p] + LR, :].rearrange("c h w -> h c w"),
            )
            nft = nfp.tile([TH, 2, W], f32r, tag="nft")
            nc.sync.dma_start(
                nft,
                nf_d[i, :, TH * p : TH * (p + 1), :].rearrange("c h w -> h c w"),
            )
            return (imgt, nft)

        def grayblock(t, ft):
            imgt, nft = ft
            i, p = divmod(t, NPOS)
            # gray_ext, halo rows at partitions 120/121; halves overlap by 2
            # (fp32r moving size must be even): A = gray -1..320, B = 319..640
            gray_ps = psum.tile([LR, 2, 512], f32, tag="gray")
            for c3 in range(3):
                for b in range(2):
                    nc.tensor.matmul(
                        gray_ps[:, b, 0:322],
                        gmat(p),
                        imgt[:, c3, b * 320 : b * 320 + 322],
                        start=(c3 == 0),
                        stop=(c3 == 2),
                    )
            # two-part drain produces the zero-padded graypad [-1..640]
            graypad = work.tile([LR, W + 2], f16, tag="graypad")
            nc.scalar.copy(graypad[:, 0:322], gray_ps[:, 0, 0:322])
            nc.scalar.copy(graypad[:, 322 : W + 2], gray_ps[:, 1, 2:322])
            # gx: 16-bit column-shift subtract on DVE (rows 0..119 = band)
            gx = work.tile([TH, W], f16, tag="gx")
            nc.vector.tensor_sub(
                gx, graypad[0:TH, 2 : W + 2], graypad[0:TH, 0:W]
            )
            return (imgt, nft, graypad, gx)

        def gymm(t, gt):
            imgt, nft, graypad, gx = gt
            i, p = divmod(t, NPOS)
            # gy (row stencil) on PE off f16 graypad
            gy_ps = psum.tile([TH, 2, 512], f32, tag="gy")
            for b, (n0, n1) in enumerate(NSPLIT):
                nc.tensor.matmul(
                    gy_ps[:, b, 0:320],
                    dmat(p),
                    graypad[:, 1 + n0 : 1 + n1],
                    start=True,
                    stop=True,
                )
            return (t, i, p, nft, gx, gy_ps)

        def stencil(sg):
            t, i, p, nft, gx, gy_ps = sg
            # gy drain on ACT (frees the gy banks); squares on DVE/Pool
            gyb = work.tile([TH, W], f16, tag="gyb")
            nc.scalar.copy(half(gyb), gy_ps[:, :, 0:320])
            gy2 = work.tile([TH, W], bf16, tag="gy2")
            nc.vector.tensor_mul(gy2, gyb, gyb)
            gx2 = work.tile([TH, W], bf16, tag="gx2")
            nc.gpsimd.tensor_mul(gx2, gx, gx)
            return (t, i, p, nft, gx, gx2, gyb, gy2)

        def normblock(ns):
            t, i, p, nft, gx, gx2, gyb, gy2 = ns
            n2 = work.tile([TH, W], bf16, tag="n2")
            nc.vector.tensor_add(n2, gx2, gy2)
            rinv = work.tile([TH, W], bf16, tag="rinv")
            if USE_ARS:
                # eps folded into the activation bias: 1/sqrt(n2 + EPS)
                nc.scalar.activation(
                    rinv, n2, AF.Abs_reciprocal_sqrt, bias=epsb[0:TH, :]
                )
            else:
                n2f = work.tile([TH, W], f32, tag="n2f")
                nc.vector.tensor_scalar_add(n2f, n2, EPS)
                inv2 = work.tile([TH, W], f32, tag="inv2")
                nc.vector.reciprocal_approx_fast(out=inv2, in_=n2f)
                nc.scalar.sqrt(rinv, inv2)
            return (t, i, p, nft, gx, gyb, rinv)

        def normprods(ns):
            t, i, p, nft, gx, gyb, rinv = ns
            gxn = work.tile([TH, W], bf16, tag="gxn")
            nc.vector.tensor_mul(gxn, gx, rinv)
            gyn = work.tile([TH, W], bf16, tag="gyn")
            nc.vector.tensor_mul(gyn, gyb, rinv)
            xgxn = work.tile([TH, W], bf16, tag="xgxn")
            nc.vector.tensor_mul(xgxn, gxn, X)
            xgyn = work.tile([TH, W], bf16, tag="xgyn")
            nc.vector.tensor_mul(xgyn, gyn, X)
            x2gxn = work.tile([TH, W], bf16, tag="x2gxn")
            nc.vector.tensor_mul(x2gxn, xgxn, X)
            return (t, i, p, nft, gxn, gyn, xgxn, xgyn, x2gxn)

        def backblock(st):
            t, i, p, nft, gxn, gyn, xgxn, xgyn, x2gxn = st
            # P/n = V2*xgxn - V0*gxn + (V2 y - V1) gyn
            P_ps = psum.tile([TH, 2, 512], f32, tag="P")
            psrc = [(sid(i, 0), xgxn), (sid(i, 1), gxn), (dg(i, p, 0), gyn)]
            for k, (m, src) in enumerate(psrc):
                for b, (n0, n1) in enumerate(NSPLIT):
                    nc.tensor.matmul(
                        P_ps[:, b, 0:320],
                        m,
                        src[:, n0:n1],
                        start=(k == 0),
                        stop=(k == len(psrc) - 1),
                    )
            # v = nf0 + nf1 - R/n
            #   = nf0 + nf1 + W1*x2gxn - W0y*xgxn + (W1y+W2)*xgyn
            #     + (W1 - W2y)*gxn - W0(y^2+1)*gyn
            v_ps = psum.tile([TH, 2, 512], f32, tag="V")
            vsrc = [
                (mask32, nft[:, 0, :]),
                (mask32, nft[:, 1, :]),
                (sid(i, 2), x2gxn),
                (dg(i, p, 1), xgxn),
                (dg(i, p, 2), xgyn),
                (dg(i, p, 3), gxn),
                (dg(i, p, 4), gyn),
            ]
            for k, (m, src) in enumerate(vsrc):
                for b, (n0, n1) in enumerate(NSPLIT):
                    nc.tensor.matmul(
                        v_ps[:, b, 0:320],
                        m,
                        src[:, n0:n1],
                        start=(k == 0),
                        stop=(k == len(vsrc) - 1),
                    )
            Pb = work.tile([TH, W], bf16, tag="pb")
            nc.scalar.copy(half(Pb), P_ps[:, :, 0:320])
            return (t, Pb, v_ps)

        def rhoblock(bk):
            t, Pb, v_ps = bk
            rho = rho_all[0:TH, t * W : (t + 1) * W]
            nc.vector.tensor_mul(half(rho), half(Pb), v_ps[:, :, 0:320])
            if check_mode:
                rho32 = work.tile([TH, W], f32, tag="rho32")
                nc.vector.tensor_copy(rho32, rho)
                nc.sync.dma_start(rho_d[:, t * W : (t + 1) * W], rho32)

        # --- software pipeline, 5-stage skew ---
        # iter k: rho(k) | P/v+Pb(k) | norms(k+1) | stencil(k+2) |
        #         gray(k+3) | dma(k+4); emission order makes each engine's
        #         in-order queue hit ops whose deps complete just-in-time.
        GELU_SPLIT = 0  # single end-batch gelu (mid-flight split measured slower)
        fts, gts, sgs, nss, sts, bks = {}, {}, {}, {}, {}, {}
        fts[0] = front(0)
        fts[1] = front(1)
        # image-1 stationaries arrive behind the first two tiles' data
        nc.sync.dma_start(dgt[:, NBLK * TH : 2 * NBLK * TH], dgt1_d)
        fts[2] = front(2)
        fts[3] = front(3)
        for k in range(-4, NT):
            if 0 <= k + 2 < NT and (k + 2) in gts:
                sgs[k + 2] = gymm(k + 2, gts.pop(k + 2))
            if 0 <= k + 1 < NT and (k + 1) in nss:
                nss[k + 1] = normblock(nss[k + 1])
            if 0 <= k < NT:
                bks[k] = backblock(sts.pop(k))
            if 0 <= k + 1 < NT and (k + 1) in nss:
                sts[k + 1] = normprods(nss.pop(k + 1))
            if 0 <= k < NT:
                rhoblock(bks.pop(k))
            if GELU_SPLIT and k == NT - 2 and not check_mode:
                nc.scalar.activation(
                    gelu_out[0:TH, 0 : GELU_SPLIT * W],
                    rho_all[0:TH, 0 : GELU_SPLIT * W],
                    AF.Gelu,
                    scale=-1.0,
                    accum_out=acc[0:TH, 0:1],
                )
            if k + 4 < NT and (k + 4) not in fts:
                fts[k + 4] = front(k + 4)
            if 0 <= k + 2 < NT and (k + 2) in sgs:
                nss[k + 2] = stencil(sgs.pop(k + 2))
            if 0 <= k + 3 < NT:
                gts[k + 3] = grayblock(k + 3, fts.pop(k + 3))

        # --- tail: gelu (remaining tiles) + reduce ---
        if not check_mode:
            nc.scalar.activation(
                gelu_out[0:TH, GELU_SPLIT * W :],
                rho_all[0:TH, GELU_SPLIT * W :],
                AF.Gelu,
                scale=-1.0,
                accum_out=acc[0:TH, 1:2],
            )
        accs = consts.tile([128, 1], f32)
        nc.vector.reduce_sum(
            accs[0:TH, :], acc[0:TH, 0:2], axis=mybir.AxisListType.X
        )
        out_ps = psum.tile([1, 1], f32, tag="gray")
        nc.tensor.matmul(
            out_ps, accs[0:TH, 0:1], ones_t[0:TH, :], start=True, stop=True
        )
        res = consts.tile([1, 1], f32)
        nc.scalar.copy(res, out_ps)
        nc.sync.dma_start(out_d, res)

    nc.compile()
    return nc


def _host_constants(pose_np):
    """Host-built constants. cstb/cstd shared; cstv/dgt1 per core."""
    import ml_dtypes

    # gmat_ext: img row -> gray_ext partition (cols 0..119 = band rows,
    # col 120 = halo row band_start-1, col 121 = halo row band_start+120)
    gmat = np.zeros((LR, NPOS, LR), np.float32)
    # dmat: graypad partition -> gy row: gy[j] = gray[j+1] - gray[j-1]
    dmat = np.zeros((LR, NPOS, TH), np.float32)
    for p in range(NPOS):
        for j in range(LR):
            g = TH * p + j if j < TH else (TH * p - 1 if j == TH else TH * p + TH)
            if 0 <= g <= H - 1:
                gmat[g - RS[p], p, j] = 1.0
        for j in range(TH):
            q_plus = j + 1 if j < TH - 1 else LR - 1
            q_minus = j - 1 if j >= 1 else TH
            dmat[q_plus, p, j] += 1.0
            dmat[q_minus, p, j] -= 1.0
    cstb = gmat.reshape(LR, -1)
    cstd = dmat.reshape(LR, -1).astype(np.float16)

    cstv_list = []
    dgt1_list = []
    jj = np.arange(TH, dtype=np.float32)
    for core in range(NCORES):
        vals = np.zeros((128, 2 * NBLK), np.float32)
        for i in range(BPC):
            b = core * BPC + i
            V0, V1, V2, W0, W1, W2 = [float(x) for x in pose_np[b]]
            base = i * NBLK
            for p in range(NPOS):
                yv = TH * p + jj
                o = base + p * 5
                vals[0:TH, o + 0] = V2 * yv - V1
                vals[0:TH, o + 1] = -W0 * yv
                vals[0:TH, o + 2] = W1 * yv + W2
                vals[0:TH, o + 3] = W1 - W2 * yv
                vals[0:TH, o + 4] = -W0 * (yv * yv + 1.0)
            vals[0:TH, base + 20] = V2
            vals[0:TH, base + 21] = -V0
            vals[0:TH, base + 22] = W1
        cstv_list.append(vals)
        dgt1 = np.zeros((TH, NBLK * TH), np.float32)
        for blk in range(NBLK):
            np.fill_diagonal(
                dgt1[:, blk * TH : (blk + 1) * TH], vals[0:TH, NBLK + blk]
            )
        dgt1_list.append(dgt1.astype(ml_dtypes.bfloat16))
    return cstb, cstd, cstv_list, dgt1_list


def kernel(img_pair, pose, normal_flow):
    from concourse.bass_utils import run_bass_kernel_spmd

    img_pair = np.asarray(img_pair, dtype=np.float32)
    pose = np.asarray(pose, dtype=np.float32)
    normal_flow = np.asarray(normal_flow, dtype=np.float32)

    if "nc" not in _CACHE:
        _CACHE["nc"] = _build_program()
    nc = _CACHE["nc"]

    cstb, cstd, cstv_list, dgt1_list = _host_constants(pose)
    in_maps = []
    for core in range(NCORES):
        b0 = core * BPC
        in_maps.append(
            {
                "img3": np.ascontiguousarray(img_pair[b0 : b0 + BPC, :3]),
                "nf": np.ascontiguousarray(normal_flow[b0 : b0 + BPC]),
                "cstb": cstb,
                "cstd": cstd,
                "cstv": cstv_list[core],
                "dgt1": dgt1_list[core],
            }
        )

    _CACHE["in_maps"] = in_maps
    res = run_bass_kernel_spmd(nc, in_maps, core_ids=list(range(NCORES)))
    total = np.float64(0.0)
    for r in res.results:
        total += np.float64(r["out"][0, 0])
    out = np.float32(total / (B * H * W))
    return np.asarray(out, dtype=np.float32)
